# revision 9
# baseline (speedup 1.0000x reference)
"""Trainium2 Bass kernel for multi-head self-attention.

Problem: B=4, S=2048, D=512, H=8 heads (DK=64), no mask, softmax without
max-subtraction (faithful to reference): attn = exp(s) / (sum(exp(s)) + 1e-8).

Sharding over 8 cores: core c handles batch b = c // 2 and the 4 heads
h0 = 4*(c % 2) .. h0+4 (x sharded by batch, weights column-sharded by head).

Per-core device pipeline (all matmuls in bf16, fp32 accumulate):
  1. Load x_b [2048, 512] fp32, PE-transpose to xT [d, s], cast bf16.
  2. Project qT/kT [e, s] (e on partitions) and v [s, e] (natural, augmented
     with a ones column per head so the PV matmul also produces the softmax
     denominator row).
  3. Per (head, q-half): loop k-tiles: scoresT = kT.T @ qT in PSUM (fp32),
     ACT exp (scale=1/8 folded in) -> SBUF bf16, PV matmul accumulates
     ctx_aug [65, 1024] in PSUM (row 64 = denominator).
  4. Finalize: PE-transpose ctx back to [q, e], multiply by
     1/(denom + 1e-8), stage in SBUF, DMA out.
"""

from contextlib import ExitStack

import numpy as np

import concourse.bass as bass
import concourse.tile as tile
from concourse import bacc, mybir
from concourse.bass_utils import run_bass_kernel_spmd
from concourse.masks import make_identity

F32 = mybir.dt.float32
BF16 = mybir.dt.bfloat16

B, S, D, H = 4, 2048, 512, 8
DK = D // H
SCALE = 1.0 / np.sqrt(DK)
N_CORES = 8
P = 128

HPC = H // 2          # heads per core = 4
E = HPC * DK          # per-core output width = 256
NS = S // P           # 16 s-tiles
NDC = D // P          # 4 d-chunks
NEC = E // P          # 2 e-chunks of projected heads
QH = 1024             # q processed per half
NQH = S // QH         # 2
EA = DK + 1           # 65: head context + denominator row


def _build_kernel(ctx: ExitStack, nc: bass.Bass, tc: tile.TileContext):
    x = nc.dram_tensor("x", [S, D], F32, kind="ExternalInput").ap()
    wq = nc.dram_tensor("wq", [D, E], F32, kind="ExternalInput").ap()
    wk = nc.dram_tensor("wk", [D, E], F32, kind="ExternalInput").ap()
    wv = nc.dram_tensor("wv", [D, E], F32, kind="ExternalInput").ap()
    out = nc.dram_tensor("out", [S, E], F32, kind="ExternalOutput").ap()

    const = ctx.enter_context(tc.tile_pool(name="const", bufs=1))
    xstage = ctx.enter_context(tc.tile_pool(name="xstage", bufs=3))
    persist = ctx.enter_context(tc.tile_pool(name="persist", bufs=1))
    exps = ctx.enter_context(tc.tile_pool(name="exps", bufs=4))
    fin = ctx.enter_context(tc.tile_pool(name="fin", bufs=2))
    # PSUM budget (8 banks): small 2x1 + scores 2x2 + ctx 1x2 = 8
    ps_small = ctx.enter_context(tc.tile_pool(name="ps_small", bufs=2, space="PSUM"))
    ps_scores = ctx.enter_context(tc.tile_pool(name="ps_scores", bufs=2, space="PSUM"))
    ps_ctx = ctx.enter_context(tc.tile_pool(name="ps_ctx", bufs=1, space="PSUM"))

    ident = const.tile([P, P], F32)
    make_identity(nc, ident)

    # ---- Phase A: load x, transpose to xT [d, s] (bf16) ----
    xT = persist.tile([P, NDC, S], BF16, name="xT")
    for st in range(NS):
        xs = xstage.tile([P, D], F32, tag="xs", name="xs")
        nc.sync.dma_start(xs[:], x[st * P : (st + 1) * P, :])
        tp = ps_small.tile([P, NDC, P], F32, tag="small", name="tp")
        for dc in range(NDC):
            nc.tensor.transpose(tp[:, dc, :], xs[:, dc * P : (dc + 1) * P], ident)
        nc.vector.tensor_copy(out=xT[:, :, st * P : (st + 1) * P], in_=tp[:])

    # ---- Phase B: weights [d, e] -> bf16 tiles [128, NDC, E] ----
    w_bf = {}
    for name, wap in (("wq", wq), ("wk", wk), ("wv", wv)):
        wf = xstage.tile([P, NDC, E], F32, tag="wstage", name=f"{name}f")
        nc.sync.dma_start(wf[:], wap.rearrange("(dc p) e -> p dc e", p=P))
        wb = persist.tile([P, NDC, E], BF16, tag=f"{name}b", name=f"{name}b")
        nc.vector.tensor_copy(out=wb[:], in_=wf[:])
        w_bf[name] = wb

    # ---- Phase C: projections ----
    # qT/kT: [e, s] with e on partitions (2 chunks of 128 = 4 heads)
    qT = persist.tile([P, NEC, S], BF16, name="qT")
    kT = persist.tile([P, NEC, S], BF16, name="kT")
    for dst, wname in ((qT, "wq"), (kT, "wk")):
        wb = w_bf[wname]
        for ec in range(NEC):
            for sc in range(S // 512):
                pp = ps_small.tile([P, 512], F32, tag="small", name="pp")
                for dc in range(NDC):
                    nc.tensor.matmul(
                        pp[:],
                        lhsT=wb[:, dc, ec * P : (ec + 1) * P],
                        rhs=xT[:, dc, sc * 512 : (sc + 1) * 512],
                        start=(dc == 0),
                        stop=(dc == NDC - 1),
                    )
                nc.vector.tensor_copy(
                    out=dst[:, ec, sc * 512 : (sc + 1) * 512], in_=pp[:]
                )

    # v natural [s, e] in ones-augmented per-head layout [128, h, 65]
    v_aug = persist.tile([P, NS, HPC, EA], BF16, name="v_aug")
    nc.gpsimd.memset(v_aug[:, :, :, DK], 1.0)
    wvb = w_bf["wv"]
    for st in range(NS):
        vp = ps_small.tile([P, E], F32, tag="small", name="vp")
        for dc in range(NDC):
            nc.tensor.matmul(
                vp[:],
                lhsT=xT[:, dc, st * P : (st + 1) * P],
                rhs=wvb[:, dc, :],
                start=(dc == 0),
                stop=(dc == NDC - 1),
            )
        nc.vector.tensor_copy(
            out=v_aug[:, st, :, 0:DK],
            in_=vp.rearrange("p (h e) -> p h e", e=DK),
        )

    # ---- Phase D: attention ----
    out_sb = persist.tile([P, NS, E], F32, name="out_sb")
    for h in range(HPC):
        ec = h // 2            # e-chunk of head h in qT/kT
        eo = DK * (h % 2)      # partition offset within the chunk
        for qh in range(NQH):
            ctx_ps = ps_ctx.tile([EA, QH], F32, tag="ctx", name="ctx_ps")
            for kt in range(NS):
                sc_ps = ps_scores.tile([P, QH], F32, tag="sc", name="sc_ps")
                for j in range(QH // 512):
                    nc.tensor.matmul(
                        sc_ps[:, j * 512 : (j + 1) * 512],
                        lhsT=kT[eo : eo + DK, ec, kt * P : (kt + 1) * P],
                        rhs=qT[eo : eo + DK, ec, qh * QH + j * 512 : qh * QH + (j + 1) * 512],
                        start=True,
                        stop=True,
                    )
                ex = exps.tile([P, QH], BF16, tag="ex", name="ex")
                nc.scalar.activation(
                    ex[:], sc_ps[:], mybir.ActivationFunctionType.Exp, scale=SCALE
                )
                for j in range(QH // 512):
                    nc.tensor.matmul(
                        ctx_ps[:, j * 512 : (j + 1) * 512],
                        lhsT=v_aug[:, kt, h, :],
                        rhs=ex[:, j * 512 : (j + 1) * 512],
                        start=(kt == 0),
                        stop=(kt == NS - 1),
                    )
            # finalize: transpose ctx back to [q, e], normalize
            caug = fin.tile([EA, QH], F32, tag="caug", name="caug")
            nc.vector.tensor_copy(out=caug[:], in_=ctx_ps[:])
            for half in range(2):
                pt = ps_small.tile([P, 4 * EA], F32, tag="small", name="pt")
                for j in range(4):
                    blk = 4 * half + j
                    nc.tensor.transpose(
                        pt[:, j * EA : (j + 1) * EA],
                        caug[:, blk * P : (blk + 1) * P],
                        ident[0:EA, 0:EA],
                    )
                pt3 = pt.rearrange("p (b e) -> p b e", e=EA)
                den = fin.tile([P, 4], F32, tag="den", name="den")
                nc.vector.tensor_scalar_add(den[:], pt3[:, :, DK], 1e-8)
                rec = fin.tile([P, 4], F32, tag="rec", name="rec")
                nc.vector.reciprocal(rec[:], den[:])
                for j in range(4):
                    qt = qh * 8 + 4 * half + j
                    nc.vector.tensor_scalar_mul(
                        out_sb[:, qt, h * DK : (h + 1) * DK],
                        pt3[:, j, 0:DK],
                        rec[:, j : j + 1],
                    )

    # ---- Phase E: store ----
    for st in range(NS):
        nc.sync.dma_start(out[st * P : (st + 1) * P, :], out_sb[:, st, :])


_COMPILED_NC = None


def _get_nc():
    global _COMPILED_NC
    if _COMPILED_NC is None:
        nc = bacc.Bacc(
            "TRN2", target_bir_lowering=False, debug=False, num_devices=N_CORES
        )
        with tile.TileContext(nc) as tc:
            with ExitStack() as ctx:
                _build_kernel(ctx, nc, tc)
        nc.compile()
        _COMPILED_NC = nc
    return _COMPILED_NC


def _shard_inputs(x, W_Q, W_K, W_V):
    """Per-core input maps: batch b = c//2, heads h0 = 4*(c%2)."""
    x = np.ascontiguousarray(np.asarray(x, dtype=np.float32))
    in_maps = []
    for c in range(N_CORES):
        b = c // 2
        h0 = HPC * (c % 2)
        rows = slice(h0 * DK, (h0 + HPC) * DK)
        in_maps.append(
            {
                "x": x[b],
                # torch Linear: y = x @ W.T -> project with W[rows].T [d, e]
                "wq": np.ascontiguousarray(np.asarray(W_Q)[rows].T.astype(np.float32)),
                "wk": np.ascontiguousarray(np.asarray(W_K)[rows].T.astype(np.float32)),
                "wv": np.ascontiguousarray(np.asarray(W_V)[rows].T.astype(np.float32)),
            }
        )
    return in_maps


def kernel(x, W_Q, W_K, W_V, _trace=False, _trace_kwargs=None):
    nc = _get_nc()
    in_maps = _shard_inputs(x, W_Q, W_K, W_V)
    res = run_bass_kernel_spmd(
        nc, in_maps, list(range(N_CORES)), trace=_trace, **(_trace_kwargs or {})
    )
    out = np.empty((B, S, D), dtype=np.float32)
    for c in range(N_CORES):
        b = c // 2
        h0 = HPC * (c % 2)
        out[b, :, h0 * DK : (h0 + HPC) * DK] = res.results[c]["out"]
    if _trace:
        return out, res
    return out


# revision 19
# speedup vs baseline: 1.0848x; 1.0848x over previous
"""Trainium2 Bass kernel for multi-head self-attention.

Problem: B=4, S=2048, D=512, H=8 heads (DK=64), no mask, softmax without
max-subtraction (faithful to reference): attn = exp(s) / (sum(exp(s)) + 1e-8).

Sharding over 8 cores: core c handles batch b = c // 2 and the 4 heads
h0 = 4*(c % 2) .. h0+4 (x sharded by batch, weights column-sharded by head).

Per-core device pipeline (all matmuls in bf16, fp32 accumulate):
  1. Load x_b [2048, 512] fp32, PE-transpose to xT [d, s], cast bf16.
  2. Project qT/kT [e, s] (e on partitions) and v [s, e] (natural, augmented
     with a ones column per head so the PV matmul also produces the softmax
     denominator row).
  3. Per (head, q-half): loop k-tiles: scoresT = kT.T @ qT in PSUM (fp32),
     ACT exp (scale=1/8 folded in) -> SBUF bf16, PV matmul accumulates
     ctx_aug [65, 1024] in PSUM (row 64 = denominator).
  4. Finalize: PE-transpose ctx back to [q, e], multiply by
     1/(denom + 1e-8), stage in SBUF, DMA out.
"""

from contextlib import ExitStack

import numpy as np

import concourse.bass as bass
import concourse.tile as tile
from concourse import bacc, mybir
from concourse.bass_utils import run_bass_kernel_spmd
from concourse.masks import make_identity

F32 = mybir.dt.float32
BF16 = mybir.dt.bfloat16

B, S, D, H = 4, 2048, 512, 8
DK = D // H
SCALE = 1.0 / np.sqrt(DK)
N_CORES = 8
P = 128

HPC = H // 2          # heads per core = 4
E = HPC * DK          # per-core output width = 256
NS = S // P           # 16 s-tiles
NDC = D // P          # 4 d-chunks
NEC = E // P          # 2 e-chunks of projected heads
QH = 512              # q processed per attention block
NQH = S // QH         # 4
EA = DK + 1           # 65: head context + denominator row


def _build_kernel(ctx: ExitStack, nc: bass.Bass, tc: tile.TileContext):
    x = nc.dram_tensor("x", [S, D], F32, kind="ExternalInput").ap()
    wq = nc.dram_tensor("wq", [D, E], F32, kind="ExternalInput").ap()
    wk = nc.dram_tensor("wk", [D, E], F32, kind="ExternalInput").ap()
    wv = nc.dram_tensor("wv", [D, E], F32, kind="ExternalInput").ap()
    out = nc.dram_tensor("out", [S, E], F32, kind="ExternalOutput").ap()

    const = ctx.enter_context(tc.tile_pool(name="const", bufs=1))
    xstage = ctx.enter_context(tc.tile_pool(name="xstage", bufs=3))
    persist = ctx.enter_context(tc.tile_pool(name="persist", bufs=1))
    exps = ctx.enter_context(tc.tile_pool(name="exps", bufs=3))
    fin = ctx.enter_context(tc.tile_pool(name="fin", bufs=2))
    # PSUM budget (8 banks): "sc" slots 2 x 2 banks (f32 [128,1024] pair
    # scores, also proj/transpose staging) + "ctx" slots 4 x 1 bank
    # ([65,512] f32 PV accumulators, finalize transposes) = 8 banks.
    ps_big = ctx.enter_context(tc.tile_pool(name="ps_big", bufs=2, space="PSUM"))
    ps_ctx = ctx.enter_context(tc.tile_pool(name="ps_ctx", bufs=4, space="PSUM"))

    ident = const.tile([P, P], F32)
    make_identity(nc, ident)

    # ---- Phase A: load x, transpose to xT [d, s] (bf16) ----
    xT = persist.tile([P, NDC, S], BF16, name="xT")
    for st in range(NS):
        xs = xstage.tile([P, D], F32, tag="xs", name="xs")
        nc.sync.dma_start(xs[:], x[st * P : (st + 1) * P, :])
        tp = ps_big.tile([P, NDC, P], F32, tag="sc", name="tp")
        for dc in range(NDC):
            nc.tensor.transpose(tp[:, dc, :], xs[:, dc * P : (dc + 1) * P], ident)
        nc.vector.tensor_copy(out=xT[:, :, st * P : (st + 1) * P], in_=tp[:])

    # ---- Phase B: weights [d, e] -> bf16 tiles [128, NDC, E] ----
    w_bf = {}
    for name, wap in (("wq", wq), ("wk", wk), ("wv", wv)):
        wf = xstage.tile([P, NDC, E], F32, tag="wstage", name=f"{name}f")
        nc.sync.dma_start(wf[:], wap.rearrange("(dc p) e -> p dc e", p=P))
        wb = persist.tile([P, NDC, E], BF16, tag=f"{name}b", name=f"{name}b")
        nc.vector.tensor_copy(out=wb[:], in_=wf[:])
        w_bf[name] = wb

    # ---- Phase C: projections ----
    # qT/kT: [e, s] with e on partitions (2 chunks of 128 = 4 heads)
    qT = persist.tile([P, NEC, S], BF16, name="qT")
    kT = persist.tile([P, NEC, S], BF16, name="kT")
    for dst, wname in ((qT, "wq"), (kT, "wk")):
        wb = w_bf[wname]
        for ec in range(NEC):
            for sc in range(S // 512):
                pp = ps_big.tile([P, 512], F32, tag="sc", name="pp")
                for dc in range(NDC):
                    nc.tensor.matmul(
                        pp[:],
                        lhsT=wb[:, dc, ec * P : (ec + 1) * P],
                        rhs=xT[:, dc, sc * 512 : (sc + 1) * 512],
                        start=(dc == 0),
                        stop=(dc == NDC - 1),
                    )
                nc.vector.tensor_copy(
                    out=dst[:, ec, sc * 512 : (sc + 1) * 512], in_=pp[:]
                )

    # v natural [s, e] in ones-augmented per-head layout [128, h, 65]
    v_aug = persist.tile([P, NS, HPC, EA], BF16, name="v_aug")
    nc.gpsimd.memset(v_aug[:, :, :, DK], 1.0)
    wvb = w_bf["wv"]
    for st in range(NS):
        vp = ps_big.tile([P, E], F32, tag="sc", name="vp")
        for dc in range(NDC):
            nc.tensor.matmul(
                vp[:],
                lhsT=xT[:, dc, st * P : (st + 1) * P],
                rhs=wvb[:, dc, :],
                start=(dc == 0),
                stop=(dc == NDC - 1),
            )
        nc.vector.tensor_copy(
            out=v_aug[:, st, :, 0:DK],
            in_=vp.rearrange("p (h e) -> p h e", e=DK),
        )

    # ---- Phase D: attention, two heads (one qT/kT partition pair) at a time ----
    out_sb = persist.tile([P, NS, E], F32, name="out_sb")
    for ec in range(NEC):      # head pair (2*ec, 2*ec+1)
        for qh in range(NQH):
            ctx_a = ps_ctx.tile([EA, QH], F32, tag="ctx", name="ctx_a")
            ctx_b = ps_ctx.tile([EA, QH], F32, tag="ctx", name="ctx_b")
            for kt in range(NS):
                # scores for both heads, f32 psum [128, 2*QH]; the two
                # matmuls occupy PE row groups 0-63 / 64-127 concurrently
                sc_ps = ps_big.tile([P, 2 * QH], F32, tag="sc", name="sc_ps")
                for hb in range(2):
                    nc.tensor.matmul(
                        sc_ps[:, hb * QH : (hb + 1) * QH],
                        lhsT=kT[hb * DK : (hb + 1) * DK, ec, kt * P : (kt + 1) * P],
                        rhs=qT[hb * DK : (hb + 1) * DK, ec, qh * QH : (qh + 1) * QH],
                        start=True,
                        stop=True,
                    )
                ex = exps.tile([P, 2 * QH], BF16, tag="ex", name="ex")
                nc.scalar.activation(
                    ex[:], sc_ps[:], mybir.ActivationFunctionType.Exp, scale=SCALE
                )
                for hb, ctx_ps in ((0, ctx_a), (1, ctx_b)):
                    nc.tensor.matmul(
                        ctx_ps[:],
                        lhsT=v_aug[:, kt, 2 * ec + hb, :],
                        rhs=ex[:, hb * QH : (hb + 1) * QH],
                        start=(kt == 0),
                        stop=(kt == NS - 1),
                    )
            # finalize both heads: transpose ctx back to [q, e], normalize
            for hb, ctx_ps in ((0, ctx_a), (1, ctx_b)):
                h = 2 * ec + hb
                caug = fin.tile([EA, QH], F32, tag="caug", name="caug")
                nc.vector.tensor_copy(out=caug[:], in_=ctx_ps[:])
                pt = ps_ctx.tile([P, 4 * EA], F32, tag="ctx", name="pt")
                for j in range(QH // P):
                    nc.tensor.transpose(
                        pt[:, j * EA : (j + 1) * EA],
                        caug[:, j * P : (j + 1) * P],
                        ident[0:EA, 0:EA],
                    )
                pt3 = pt.rearrange("p (b e) -> p b e", e=EA)
                den = fin.tile([P, 4], F32, tag="den", name="den")
                nc.vector.tensor_scalar_add(den[:], pt3[:, :, DK], 1e-8)
                rec = fin.tile([P, 4], F32, tag="rec", name="rec")
                nc.vector.reciprocal(rec[:], den[:])
                for j in range(QH // P):
                    qt = qh * (QH // P) + j
                    nc.vector.tensor_scalar_mul(
                        out_sb[:, qt, h * DK : (h + 1) * DK],
                        pt3[:, j, 0:DK],
                        rec[:, j : j + 1],
                    )

    # ---- Phase E: store ----
    for st in range(NS):
        nc.sync.dma_start(out[st * P : (st + 1) * P, :], out_sb[:, st, :])


_COMPILED_NC = None


def _get_nc():
    global _COMPILED_NC
    if _COMPILED_NC is None:
        nc = bacc.Bacc(
            "TRN2", target_bir_lowering=False, debug=False, num_devices=N_CORES
        )
        with tile.TileContext(nc) as tc:
            with ExitStack() as ctx:
                _build_kernel(ctx, nc, tc)
        nc.compile()
        _COMPILED_NC = nc
    return _COMPILED_NC


def _shard_inputs(x, W_Q, W_K, W_V):
    """Per-core input maps: batch b = c//2, heads h0 = 4*(c%2)."""
    x = np.ascontiguousarray(np.asarray(x, dtype=np.float32))
    in_maps = []
    for c in range(N_CORES):
        b = c // 2
        h0 = HPC * (c % 2)
        rows = slice(h0 * DK, (h0 + HPC) * DK)
        in_maps.append(
            {
                "x": x[b],
                # torch Linear: y = x @ W.T -> project with W[rows].T [d, e]
                "wq": np.ascontiguousarray(np.asarray(W_Q)[rows].T.astype(np.float32)),
                "wk": np.ascontiguousarray(np.asarray(W_K)[rows].T.astype(np.float32)),
                "wv": np.ascontiguousarray(np.asarray(W_V)[rows].T.astype(np.float32)),
            }
        )
    return in_maps


def kernel(x, W_Q, W_K, W_V, _trace=False, _trace_kwargs=None):
    nc = _get_nc()
    in_maps = _shard_inputs(x, W_Q, W_K, W_V)
    res = run_bass_kernel_spmd(
        nc, in_maps, list(range(N_CORES)), trace=_trace, **(_trace_kwargs or {})
    )
    out = np.empty((B, S, D), dtype=np.float32)
    for c in range(N_CORES):
        b = c // 2
        h0 = HPC * (c % 2)
        out[b, :, h0 * DK : (h0 + HPC) * DK] = res.results[c]["out"]
    if _trace:
        return out, res
    return out
